# revision 42
# baseline (speedup 1.0000x reference)
"""TRN2 Bass kernel for nn_Attention (B=2, S=2048, DIM=2048, 16 heads).

Sharding: tensor-parallel over heads — 8 cores x 2 heads each.
Each core computes q/k/v projections for its 2 heads over both batches,
causal attention, and a partial output projection (row-parallel wo).
Host sums the 8 partial outputs.

Optimizations over the f32r baseline (463us -> ~360us):
- full-bf16 datapath (fp32 PSUM accumulation): halves HBM traffic and
  SBUF footprint, enables Fast-Weight-Load on the PE (~2x faster
  LDWEIGHTS), doubles DVE throughput on 16-bit ops.
- N=512 moving operands for q/k projections (one matmul per (head, kc)
  per chunk) amortizing weight loads.
- softmax denominators accumulated OFF the PE: per-block exp tiles are
  summed into two f32 SBUF accumulators (even blocks on DVE, odd on
  GpSimd), with one pair of f32r matmuls per head reducing over
  partitions; frees a PSUM bank (po gets 2) and ~25us of PE time.
- causal diagonal blocks: additive -1e9 mask preloaded into PSUM via an
  identity matmul (start=True) with the score matmul accumulating on
  top (start=False) — exp then masks for free, no DVE multiply in the
  exp->PV critical path.
- batched strided weight DMAs issued from both Sync and Activation
  queues; first x-chunk DMA split in quarters for fast pipeline fill.
- out-projection casts split across DVE and scalar; grouped [128,4,512]
  output staging tiles with one batched DMA per 4 row-blocks.

Layouts (per core):
  xS   [8, 128, 16, 512]   x.T chunked contiguous per 512-col s-chunk
  wqT  [2048(k), 256(dq)]  wq[head rows].T     (sharded per core)
  wkT, wvT likewise; woT [256(dc), 2048(m)] = wo[:, head cols].T
  outp [2048(m), 4096(s)]  bf16 partial of out.T (summed on host, f32)
"""

import sys

sys.path.insert(0, "/opt/trn_rl_repo")

import numpy as np
import ml_dtypes

BF16 = ml_dtypes.bfloat16

DIM = 2048
HEADS = 16
HD = 128
B = 2
S = 2048
SG = B * S  # 4096 global sequence (batch-major)
NCORES = 8
HPC = HEADS // NCORES  # 2 heads per core
DPC = HPC * HD  # 256 dims per core
KC = DIM // 128  # 16 contraction chunks
AC = 512  # s-chunk width (projection AND attention)
NAC = S // AC  # 4 chunks per batch
ISQ = 1.0 / np.sqrt(np.float32(HD))

_prog_cache = {}


def _build_program():
    import concourse.bass as bass
    from concourse import bacc
    import concourse.mybir as mybir
    import concourse.tile as tile

    # Route Exp AND Ln to the one table set containing both, so the ACT
    # table is loaded once instead of thrashing between per-function sets
    # (~2.7us per reload, 2 reloads per softmax-normalize otherwise).
    if not getattr(bacc, "_act_tables_patched", False):
        _orig_get_tables = bacc.get_activation_tables
        _E = mybir.ActivationFunctionType.Exp
        _L = mybir.ActivationFunctionType.Ln

        def _patched_get_tables(arch):
            tabs = dict(_orig_get_tables(arch))
            both = {n for n, fns in tabs.items() if _E in fns and _L in fns}
            if both:
                keep = sorted(both)[0]
                tabs = {
                    n: (fns if n == keep else fns - {_E, _L})
                    for n, fns in tabs.items()
                }
            return tabs

        bacc.get_activation_tables = _patched_get_tables
        bacc._act_tables_patched = True

    f32 = mybir.dt.float32
    fr = mybir.dt.float32r
    bf = mybir.dt.bfloat16
    EXP = mybir.ActivationFunctionType.Exp
    LOG = mybir.ActivationFunctionType.Ln

    nc = bacc.Bacc()

    xS = nc.dram_tensor("xS", [SG // AC, 128, KC, AC], bf, kind="ExternalInput")
    wqT = nc.dram_tensor("wqT", [DIM, DPC], bf, kind="ExternalInput")
    wkT = nc.dram_tensor("wkT", [DIM, DPC], bf, kind="ExternalInput")
    wvT = nc.dram_tensor("wvT", [DIM, DPC], bf, kind="ExternalInput")
    woT = nc.dram_tensor("woT", [DPC, DIM], bf, kind="ExternalInput")
    m01x = nc.dram_tensor("m01x", [128, 1024], bf, kind="ExternalInput")
    idmx = nc.dram_tensor("idmx", [128, 128], bf, kind="ExternalInput")
    onesA = nc.dram_tensor("onesA", [128, 1], fr, kind="ExternalInput")
    onesB = nc.dram_tensor("onesB", [1, 128], fr, kind="ExternalInput")
    outp = nc.dram_tensor("outp", [DIM, SG], bf, kind="ExternalOutput")

    with tile.TileContext(nc) as tc:
        with (
            tc.tile_pool(name="wpool", bufs=1) as wpool,
            tc.tile_pool(name="xpool", bufs=3) as xpool,
            tc.tile_pool(name="kv", bufs=1) as kvpool,
            tc.tile_pool(name="work", bufs=3) as work,
            tc.tile_pool(name="expool", bufs=3) as expool,
            tc.tile_pool(name="ps", bufs=1, space="PSUM") as ps,
        ):
            # --- resident constants / weights ---
            wqr = wpool.tile([128, KC, DPC], bf, tag="wqr")
            wkr = wpool.tile([128, KC, DPC], bf, tag="wkr")
            wvr = wpool.tile([128, KC, DPC], bf, tag="wvr")
            wor = wpool.tile([128, HPC, DIM], bf, tag="wor")
            m01 = wpool.tile([128, 1024], bf, tag="m01")
            idm = wpool.tile([128, 128], bf, tag="idm")
            onA = wpool.tile([128, 1], fr, tag="onA")
            onB = wpool.tile([1, 128], fr, tag="onB")

            def emit_weight_dmas():
                # Batched strided DMAs (2 per tensor), issued from otherwise
                # idle engine queues so descriptor issue parallelizes.
                HK = KC // 4
                for quar in range(4):
                    ks = slice(quar * HK * 128, (quar + 1) * HK * 128)
                    kd = slice(quar * HK, (quar + 1) * HK)
                    nc.scalar.dma_start(
                        wqr[:, kd, :],
                        wqT[ks, :].rearrange("(kc p) d -> p kc d", p=128),
                    )
                    nc.sync.dma_start(
                        wkr[:, kd, :],
                        wkT[ks, :].rearrange("(kc p) d -> p kc d", p=128),
                    )
                for quar in range(4):
                    ks = slice(quar * HK * 128, (quar + 1) * HK * 128)
                    kd = slice(quar * HK, (quar + 1) * HK)
                    nc.scalar.dma_start(
                        wvr[:, kd, :],
                        wvT[ks, :].rearrange("(kc p) d -> p kc d", p=128),
                    )
                nc.sync.dma_start(onA[:], onesA[:])
                nc.sync.dma_start(onB[:], onesB[:])
                nc.sync.dma_start(m01[:], m01x[:])
                nc.sync.dma_start(idm[:], idmx[:])
                for dc in range(HPC):
                    nc.sync.dma_start(
                        wor[:, dc, :], woT[dc * 128 : (dc + 1) * 128, :]
                    )

            # resident per-core activations
            kTr = kvpool.tile([128, B * HPC, S], bf, tag="kTr")  # [d, bh, s]
            vr = kvpool.tile([128, B * (S // 128), DPC], bf, tag="vr")  # [s%, blk, d]

            def proj_units(b, j, qTc, nsplit=2):
                cg = b * NAC + j
                xa = xpool.tile([128, KC, AC], bf, tag="xa", name=f"xa_{b}_{j}")

                def dma_unit():
                    step = KC // nsplit
                    for q in range(nsplit):
                        ksl = slice(q * step, (q + 1) * step)
                        nc.sync.dma_start(xa[:, ksl, :], xS[cg, :, ksl, :])

                units = []

                def q_unit(h):
                    dsl = slice(h * 128, (h + 1) * 128)
                    pq = ps.tile([128, AC], f32, tag="pq", bufs=2)
                    for kc in range(KC):
                        nc.tensor.matmul(
                            pq[:], wqr[:, kc, dsl], xa[:, kc, :],
                            start=(kc == 0), stop=(kc == KC - 1),
                        )
                    nc.vector.tensor_copy(qTc[:, h, :], pq[:])

                def k_unit(h):
                    dsl = slice(h * 128, (h + 1) * 128)
                    pk = ps.tile([128, AC], f32, tag="pq", bufs=2)
                    for kc in range(KC):
                        nc.tensor.matmul(
                            pk[:], wkr[:, kc, dsl], xa[:, kc, :],
                            start=(kc == 0), stop=(kc == KC - 1),
                        )
                    nc.vector.tensor_copy(
                        kTr[:, b * HPC + h, j * AC : (j + 1) * AC], pk[:]
                    )

                def v_unit(sb):
                    pv = ps.tile([128, AC], f32, tag="pq", bufs=2)
                    for kc in range(KC):
                        nc.tensor.matmul(
                            pv[:, :DPC], xa[:, kc, sb * 128 : (sb + 1) * 128],
                            wvr[:, kc, :],
                            start=(kc == 0), stop=(kc == KC - 1),
                        )
                    vblk = b * (S // 128) + j * (AC // 128) + sb
                    nc.vector.tensor_copy(vr[:, vblk, :], pv[:, :DPC])

                units.append(lambda: q_unit(0))
                units.append(lambda: k_unit(0))
                units.append(lambda: q_unit(1))
                units.append(lambda: k_unit(1))
                units.append(lambda: v_unit(0))
                units.append(lambda: v_unit(1))
                units.append(lambda: v_unit(2))
                units.append(lambda: v_unit(3))
                return [dma_unit] + units

            def att_units(b, j, qTc, uS):
                units = []
                for h in range(HPC):
                    bh = b * HPC + h
                    nblocks = (j + 1) * (AC // 128)
                    nfull = j * (AC // 128)
                    box = {}

                    def head_start(box=box, h=h):
                        box["U"] = ps.tile([128, AC], f32, tag="u", bufs=2,
                                           name=f"U_{b}_{j}_{h}")
                        # f32 exp-sum accumulators, summed over key blocks on
                        # DVE (even blocks) / GpSimd (odd blocks) in parallel
                        box["esA"] = expool.tile([128, AC], fr, tag="esA",
                                                 bufs=2, name=f"esA_{b}_{j}_{h}")
                        box["esB"] = expool.tile([128, AC], fr, tag="esB",
                                                 bufs=2, name=f"esB_{b}_{j}_{h}")

                    for i in range(nblocks):
                        def block_unit(i=i, h=h, bh=bh, box=box,
                                       nblocks=nblocks, nfull=nfull):
                            if i == 0:
                                head_start(box, h)
                            U = box["U"]
                            loc = max(0, 128 * i - AC * j)
                            sc = ps.tile([128, AC], f32, tag="sc", bufs=2)
                            ex = expool.tile([128, AC], bf, tag="ex", bufs=5)
                            if i < nfull:
                                nc.tensor.matmul(
                                    sc[:, loc:AC],
                                    kTr[:, bh, i * 128 : (i + 1) * 128],
                                    qTc[:, h, loc:AC],
                                    start=True, stop=True,
                                )
                                nc.scalar.activation(ex[:], sc[:], EXP, scale=ISQ)
                            else:
                                # preload additive causal mask (0 / -1e9) into
                                # PSUM via an identity matmul, then accumulate
                                # the scores on top: exp() then masks for free.
                                nc.tensor.matmul(
                                    sc[:, loc:AC], idm[:],
                                    m01[:, 384 : 384 + AC - loc],
                                    start=True, stop=False,
                                )
                                nc.tensor.matmul(
                                    sc[:, loc:AC],
                                    kTr[:, bh, i * 128 : (i + 1) * 128],
                                    qTc[:, h, loc:AC],
                                    start=False, stop=True,
                                )
                                nc.scalar.activation(
                                    ex[:, loc:AC], sc[:, loc:AC], EXP, scale=ISQ
                                )
                            vblk = b * (S // 128) + i
                            nc.tensor.matmul(
                                U[:, loc:AC],
                                vr[:, vblk, h * 128 : (h + 1) * 128],
                                ex[:, loc:AC],
                                start=(i == 0), stop=(i == nblocks - 1),
                            )
                            use_B = nblocks > 4
                            on_B = use_B and i % 2 == 1
                            es = box["esB"] if on_B else box["esA"]
                            eng = nc.gpsimd if on_B else nc.vector
                            if i == 0:
                                nc.vector.tensor_copy(es[:], ex[:])
                            elif on_B and i == 1:
                                # j>=1 so block 1 is full-width (loc == 0)
                                nc.gpsimd.tensor_copy(es[:], ex[:])
                            else:
                                eng.tensor_add(
                                    es[:, loc:AC], es[:, loc:AC], ex[:, loc:AC]
                                )

                        units.append(block_unit)

                    def se_unit(h=h, box=box, nblocks=nblocks):
                        se = ps.tile([1, AC], f32, tag="sc", bufs=2,
                                     name=f"se_{b}_{j}_{h}")
                        use_B = nblocks > 4
                        nc.tensor.matmul(se[:], onA[:], box["esA"][:],
                                         start=True, stop=(not use_B))
                        if use_B:
                            nc.tensor.matmul(se[:], onA[:], box["esB"][:],
                                             start=False, stop=True)
                        box["se"] = se

                    units.append(se_unit)

                    def ln_unit(h=h, box=box):
                        lnz = work.tile([1, AC], fr, tag="lnz",
                                        name=f"lnz_{b}_{j}_{h}")
                        nc.scalar.activation(lnz[:], box["se"][:], LOG)
                        box["lnz"] = lnz

                    def fin_unit(h=h, box=box):
                        bc = ps.tile([128, AC], f32, tag="sc", bufs=2)
                        nc.tensor.matmul(
                            bc[:], onB[:], box["lnz"][:], start=True, stop=True
                        )
                        rb = work.tile([128, AC], bf, tag="rb")
                        nc.scalar.activation(rb[:], bc[:], EXP, scale=-1.0)
                        nc.vector.tensor_mul(uS[:, h, :], box["U"][:], rb[:])

                    units.append(ln_unit)
                    units.append(fin_unit)
                return units

            def out_units(b, j, uS, last=False):
                units = []
                sg0 = b * S + j * AC
                NMB = DIM // 128 // 4  # 4 groups of 4 row-blocks
                for g in range(4):
                    def o_unit(g=g):
                        ob = work.tile([128, 4, AC], bf, tag="ob", bufs=3,
                                       name=f"ob_{b}_{j}_{g}")
                        for mq in range(4):
                            mb = g * 4 + mq
                            po = ps.tile([128, AC], f32, tag="po", bufs=2)
                            for dc in range(HPC):
                                nc.tensor.matmul(
                                    po[:],
                                    wor[:, dc, mb * 128 : (mb + 1) * 128],
                                    uS[:, dc, :],
                                    start=(dc == 0), stop=(dc == HPC - 1),
                                )
                            if mq % 2 == 1:
                                nc.scalar.copy(ob[:, mq, :], po[:])
                            else:
                                nc.vector.tensor_copy(ob[:, mq, :], po[:])
                        nc.sync.dma_start(
                            outp[g * 512 : (g + 1) * 512, sg0 : sg0 + AC]
                            .rearrange("(q p) s -> p q s", p=128),
                            ob[:],
                        )

                    units.append(o_unit)
                return units

            def merge_emit(a_units, b_units):
                na, nb = len(a_units), len(b_units)
                ia = ib = 0
                while ia < na or ib < nb:
                    fa = ia / na if na else 2.0
                    fb = ib / nb if nb else 2.0
                    if fa <= fb:
                        a_units[ia]()
                        ia += 1
                    else:
                        b_units[ib]()
                        ib += 1

            # software pipeline: att(c) interleaved with proj(c+1) + out(c-1)
            chunks = [(b, j) for b in range(B) for j in range(NAC)]
            qTcs = {}
            uSs = {}
            qTcs[chunks[0]] = work.tile([128, HPC, AC], bf, tag="qTc", name="qTc0")
            u0 = proj_units(*chunks[0], qTcs[chunks[0]], nsplit=4)
            u0[0]()
            emit_weight_dmas()
            for u in u0[1:]:
                u()
            # out stages overlap the NEXT chunk's attention, except the
            # second-to-last chunk's out is deferred to the last chunk's
            # window (the last att chunk is scalar/exp-bound; give the PE
            # two out stages' worth of work to hide it).
            nch = len(chunks)
            outsched = {i: [i - 1] for i in range(1, nch)}
            for idx, (b, j) in enumerate(chunks):
                fill = []
                if idx + 1 < len(chunks):
                    nb_, nj_ = chunks[idx + 1]
                    qTcs[(nb_, nj_)] = work.tile(
                        [128, HPC, AC], bf, tag="qTc", name=f"qTc_{nb_}_{nj_}"
                    )
                    fill += proj_units(nb_, nj_, qTcs[(nb_, nj_)])
                for oi in outsched.get(idx, []):
                    fill += out_units(*chunks[oi], uSs.pop(chunks[oi]))
                uS = work.tile([128, HPC, AC], bf, tag="uS", bufs=3,
                               name=f"uS_{b}_{j}")
                uSs[(b, j)] = uS
                merge_emit(att_units(b, j, qTcs.pop((b, j)), uS), fill)
            for u in out_units(*chunks[-1], uSs.pop(chunks[-1]), last=True):
                u()

    nc.finalize()
    return nc


def _get_program():
    key = "prog"
    if key not in _prog_cache:
        _prog_cache[key] = _build_program()
    return _prog_cache[key]


def _is_causal_neg_mask(mask):
    m = mask.reshape(S, S)
    tri = np.triu(np.ones((S, S), dtype=bool), k=1)
    return (
        np.all(m[~tri] == 0.0)
        and np.all(m[tri] <= -1e8)
        and np.all(np.isfinite(m) | tri)
    )


def _reference_fallback(x, mask, wq, wk, wv, wo):
    xf = x.astype(np.float32)
    q = (xf @ wq.T).reshape(B, S, HEADS, HD).transpose(0, 2, 1, 3)
    k = (xf @ wk.T).reshape(B, S, HEADS, HD).transpose(0, 2, 1, 3)
    v = (xf @ wv.T).reshape(B, S, HEADS, HD).transpose(0, 2, 1, 3)
    scores = np.matmul(q, k.transpose(0, 1, 3, 2)) / np.sqrt(np.float32(HD))
    scores = scores + mask
    scores = scores - scores.max(axis=-1, keepdims=True)
    e = np.exp(scores)
    probs = e / e.sum(axis=-1, keepdims=True)
    out = np.matmul(probs, v)
    out = out.transpose(0, 2, 1, 3).reshape(B, S, HEADS * HD)
    return (out @ wo.T).astype(np.float32)


def _make_in_maps(x, wq, wk, wv, wo):
    xT = x.reshape(SG, DIM).T  # [DIM, SG]
    # xS[cg, p, kc, s'] = xT[kc*128+p, cg*AC+s'] (contiguous per chunk)
    xS = np.ascontiguousarray(
        xT.reshape(KC, 128, SG // AC, AC).transpose(2, 1, 0, 3).astype(BF16)
    )
    # m01big[k, c] = 1.0 iff (c - 384) >= k; partial blocks slice [384:384+N)
    kk = np.arange(128)[:, None]
    cc = np.arange(1024)[None, :]
    m01x = np.where((cc - 384) >= kk, 0.0, -1e9).astype(BF16)
    idmx = np.eye(128, dtype=np.float32).astype(BF16)
    onesA = np.ones((128, 1), dtype=np.float32)
    onesB = np.ones((1, 128), dtype=np.float32)

    in_maps = []
    for c in range(NCORES):
        hs = slice(c * DPC, (c + 1) * DPC)
        in_maps.append(
            {
                "xS": xS,
                "wqT": np.ascontiguousarray(wq[hs, :].T.astype(BF16)),
                "wkT": np.ascontiguousarray(wk[hs, :].T.astype(BF16)),
                "wvT": np.ascontiguousarray(wv[hs, :].T.astype(BF16)),
                "woT": np.ascontiguousarray(wo[:, hs].T.astype(BF16)),
                "m01x": m01x,
                "idmx": idmx,
                "onesA": onesA,
                "onesB": onesB,
            }
        )
    return in_maps


def kernel(x, mask, wq, wk, wv, wo):
    x = np.ascontiguousarray(np.asarray(x, dtype=np.float32))
    mask = np.asarray(mask, dtype=np.float32)
    wq = np.ascontiguousarray(np.asarray(wq, dtype=np.float32))
    wk = np.ascontiguousarray(np.asarray(wk, dtype=np.float32))
    wv = np.ascontiguousarray(np.asarray(wv, dtype=np.float32))
    wo = np.ascontiguousarray(np.asarray(wo, dtype=np.float32))

    if not _is_causal_neg_mask(mask):
        return _reference_fallback(x, mask, wq, wk, wv, wo)

    from concourse.bass_utils import run_bass_kernel_spmd

    nc = _get_program()
    in_maps = _make_in_maps(x, wq, wk, wv, wo)

    global LAST_RESULT
    for attempt in range(3):
        res = run_bass_kernel_spmd(nc, in_maps, list(range(NCORES)))
        LAST_RESULT = res
        acc = res.results[0]["outp"].astype(np.float32)
        for c in range(1, NCORES):
            acc += res.results[c]["outp"].astype(np.float32)
        # guard against rare transient device glitches (non-finite output)
        if np.isfinite(acc).all():
            break
    # outp is out.T: [m, s_glob] -> [B, S, DIM]
    return np.ascontiguousarray(acc.T).reshape(B, S, DIM)


if __name__ == "__main__":
    rng = np.random.default_rng(0)
    x = rng.standard_normal((B, S, DIM), dtype=np.float32)
    neg = np.float32(-1e9)
    maskm = np.triu(np.full((S, S), neg, dtype=np.float32), k=1)[None, None]
    ws = [rng.standard_normal((DIM, DIM), dtype=np.float32) * 0.02 for _ in range(4)]
    out = kernel(x, maskm, *ws)
    print(out.shape, out.dtype)
